# revision 18
# baseline (speedup 1.0000x reference)
"""GCN joint-representation edge MLP on 8 TRN2 NeuronCores (Bass/Tile).

reference:
    node_rep = z[edge_index[0]] * z[edge_index[1]]          # [E, 64]
    joint    = concat([node_rep, edge_attr], -1)            # [E, 832]
    h        = relu(joint @ W1 + b1)                        # [E, 128]
    out      = softmax(h @ W2 + b2, -1)                     # [E, 5]

Sharding: pure data-parallel over edges, 8 cores x 25088 edges (E padded
200000 -> 200704).  Each core streams its edge slice and runs the full
MLP + softmax on device.

Key measured facts this design is built around (HW-probed):
  - per-dma_start cost is ~1-2us regardless of size: attr/zz streams are
    fetched in 7-block batches (2.7MB/0.9MB), prefetched one batch ahead,
    and the output leaves in ONE DMA from a resident SBUF tile.
  - engine queues are strict FIFO; every op has a ~150-400ns dispatch/sem
    floor, and a cross-engine handoff at a queue head stalls everything
    behind it.  Per-block op count is minimized, the block loop is
    software-pipelined 4 deep so every op only consumes data >= 1
    iteration old, and the softmax runs batched per 2-block group.
  - a PE ldweights(128 cols) is only free when the PREVIOUS matmul
    streams >= 128 cycles.  Layer 2 is edge-major (lhsT = hT 128-edge
    chunk, a fresh stationary operand each time), so W2 is zero-padded
    from [128, 5] to [128, 128]: each chunk matmul then streams N=128 and
    the next chunk's weight load hides completely.  With N=5 streams the
    same 4 matmuls measured 400ns each; padded they pipeline at ~60-150ns.

Layout choices made during host-side sharding:
  - endpoint z-rows are resolved to dense per-edge streams (z[src], z[dst]).
    Device-side row-gather primitives are unusable in this runtime
    (multi-offset indirect DMA returns wrong data on HW; the dma_gather
    GPSIMD ucode crashes the exec unit; per-128-row indirect DMA costs
    1.6us/call).  The dense streams carry byte-for-byte the same device
    traffic as an on-device gather would.
  - edge_attr is fp8 e4m3 (TRN FP8_EXP4), W1's attr rows are fp8 scaled
    x64 into the format's normal range; layer 1 contracts them with
    3 DoubleRow matmuls (2 fp8 K-tiles per pass) and the relu activation
    descales via scale=1/64.  z / node_rep stay bf16.  Measured rel err
    1.35e-2 vs the f32 reference (tolerance 2e-2).
  - attr is packed [128, nblk, 6, 512] so each batch DMA is contiguous
    per partition; probs leave edge-major ([p, b, ch, class], edge =
    b*512 + ch*128 + p) as bf16 in one final DMA.

Per 512-edge block: SP attr DMA ~1.1us (amortized), POOL zz DMA (SWDGE)
~0.6us, DVE nr mul ~0.4us, PE 4 layer-1 matmuls + 4 padded layer-2
matmuls ~1.4us, ACT relu ~0.7us; per 2-block group: ACT exp [128,8,5],
DVE class-sum reduce + reciprocal + broadcast multiply (~0.5us total).
"""
import numpy as np

import concourse.bass as bass
import concourse.bacc as bacc
import concourse.tile as tile
from concourse import mybir
from concourse.bass_utils import run_bass_kernel_spmd

F32 = mybir.dt.float32
F32R = mybir.dt.float32r
BF16 = mybir.dt.bfloat16
F8 = mybir.dt.float8e4
W1SC = 64.0                 # W1 is sent scaled by 64 (fp8 normal range);
                            # the relu activation descales via scale=1/64

N_CORES = 8
E_FULL = 200000
E_PAD = 200704              # 8 * 25088
E_CORE = E_PAD // N_CORES   # 25088 = 49 * 512
BLK = 512
NBLK = E_CORE // BLK        # 49
ZD = 64
AD = 768
NSL = AD // 128             # 6 attr feature slices
HID = 128
NCLS = 5
NCH = BLK // 128            # 4 edge chunks per block for layer 2
W2P = 128                   # W2 padded N (stream long enough to hide LDW)
GRP = 2                     # blocks per softmax batch (PSUM: 2 banks/group)
DMAB = 7                    # blocks per attr/zz DMA batch (49 = 7*7)


def build_nc(nblk=NBLK, reps=1, mode="full"):
    """Per-core Bass program (same NEFF on all 8 cores).  `reps` wraps the
    block loop with a For_i for timing runs.  mode: full | dma | l1
    (bisection probes)."""
    nc = bacc.Bacc("TRN2", target_bir_lowering=False, debug=False)

    ecore = nblk * BLK
    attrP = nc.declare_dram_parameter("attrP", [128, nblk * NSL * BLK], F8,
                                      isOutput=False)
    zzP = nc.declare_dram_parameter("zzP", [ZD, 2 * ecore], BF16, isOutput=False)
    w1a = nc.declare_dram_parameter("w1a", [ZD, HID], BF16, isOutput=False)
    w1f = nc.declare_dram_parameter("w1f", [128, NSL, HID], F8, isOutput=False)
    w2p = nc.declare_dram_parameter("w2p", [HID, W2P], F32, isOutput=False)
    b1 = nc.declare_dram_parameter("b1", [HID, 1], F32, isOutput=False)
    eb2 = nc.declare_dram_parameter("eb2", [128, NCLS], BF16, isOutput=False)
    # edge-major probs: [p, b*NCH*NCLS] with edge = b*512 + ch*128 + p
    outE = nc.declare_dram_parameter("outE", [128, nblk * NCH * NCLS], BF16,
                                     isOutput=True)

    assert nblk % DMAB == 0 or nblk < DMAB
    dmab = min(DMAB, nblk)
    nfg = nblk // dmab
    attrP_v = attrP[:, :].rearrange("p (B d s e) -> B p d s e",
                                    d=dmab, s=NSL, e=BLK)
    zzP_v = zzP[:, :].rearrange("p (B d e) -> B p d e", d=dmab, e=2 * BLK)

    # softmax groups: (first block, #blocks)
    sgs = [(s, min(GRP, nblk - s)) for s in range(0, nblk, GRP)]
    sg_of_end = {s + n - 1: (i, s, n) for i, (s, n) in enumerate(sgs)}

    with tile.TileContext(nc) as tc:
        with (
            tc.tile_pool(name="const", bufs=1) as constp,
            tc.tile_pool(name="attrp", bufs=3) as attrp,
            tc.tile_pool(name="zp", bufs=3) as zp,
            tc.tile_pool(name="nrp", bufs=3) as nrp,
            tc.tile_pool(name="htp", bufs=2) as htp,
            tc.tile_pool(name="exg", bufs=2) as exgp,
            tc.tile_pool(name="recg", bufs=2) as recgp,
            tc.tile_pool(name="ps_ht", bufs=2, space="PSUM") as ps_ht,
            tc.tile_pool(name="ps_lg", bufs=2, space="PSUM") as ps_lg,
        ):
            # ---- constants ----
            w1a_t = constp.tile([ZD, HID], BF16)
            nc.sync.dma_start(out=w1a_t[:], in_=w1a[:, :])
            w1f_t = constp.tile([128, NSL, HID], F8)
            nc.sync.dma_start(out=w1f_t[:], in_=w1f[:, :, :])
            w2p_t = constp.tile([HID, W2P], F32R)
            nc.sync.dma_start(out=w2p_t[:], in_=w2p[:, :].bitcast(F32R))
            b1_t = constp.tile([HID, 1], F32)
            nc.sync.dma_start(out=b1_t[:], in_=b1[:, :])
            eb2_t = constp.tile([128, NCLS], BF16)
            nc.sync.dma_start(out=eb2_t[:], in_=eb2[:, :])
            out_t = constp.tile([128, nblk * NCH * NCLS], BF16)
            if mode in ("dma", "l1"):
                nc.vector.memset(out_t[:], 0.125)

            live = {}

            def fetch(B):
                if B >= nfg:
                    return
                attr_g = attrp.tile([128, dmab, NSL, BLK], F8, tag="attr")
                nc.sync.dma_start(out=attr_g[:], in_=attrP_v[B])
                zz_g = zp.tile([ZD, dmab, 2 * BLK], BF16, tag="zz")
                nc.gpsimd.dma_start(out=zz_g[:], in_=zzP_v[B])
                live[("attr_g", B)] = attr_g
                live[("zz_g", B)] = zz_g

            def stageNR(b):
                # node_rep for block b, one iteration before its matmuls
                B, d = divmod(b, dmab)
                zz_t = live[("zz_g", B)][:, d]
                nr_t = nrp.tile([ZD, BLK], BF16, tag="nr")
                nc.vector.tensor_mul(nr_t[:], zz_t[:, 0:BLK], zz_t[:, BLK:2 * BLK])
                live[("nr", b)] = nr_t

            def stageA(b):
                B, d = divmod(b, dmab)
                if d == 0:
                    fetch(B + 1)
                attr_t = live[("attr_g", B)][:, d]
                nr_t = live.pop(("nr", b))
                ht_ps = ps_ht.tile([HID, BLK], F32, tag="htps")
                nc.tensor.matmul(out=ht_ps[:], lhsT=w1a_t[:], rhs=nr_t[:],
                                 start=True, stop=False)
                for s in range(NSL // 2):
                    nc.tensor.matmul(out=ht_ps[:], lhsT=w1f_t[:, 2 * s:2 * s + 2, :],
                                     rhs=attr_t[:, 2 * s:2 * s + 2, :],
                                     perf_mode=mybir.MatmulPerfMode.DoubleRow,
                                     start=False, stop=(s == NSL // 2 - 1))
                live[("ht_ps", b)] = ht_ps

            def stageB(b):
                # relu for block b, then 4 edge-major padded layer-2 matmuls
                ht_ps = live.pop(("ht_ps", b))
                ht_s = htp.tile([HID, BLK], F32R, tag="hts")
                nc.scalar.activation(out=ht_s[:], in_=ht_ps[:],
                                     func=mybir.ActivationFunctionType.Relu,
                                     bias=b1_t[:], scale=1.0 / W1SC)
                if mode == "l1":
                    return
                g, s, n = sg_of_end.get(b, (None, None, None))
                if b % GRP == 0:
                    lg_ps = ps_lg.tile([128, GRP * NCH, W2P], F32, tag="lgps")
                    live[("lg_ps", b // GRP)] = lg_ps
                lg_ps = live[("lg_ps", b // GRP)]
                j = b % GRP
                for c in range(NCH):
                    # f32r lhsT -> single self-loading matmul (no ldweights)
                    nc.tensor.matmul(
                        out=lg_ps[:, j * NCH + c, :],
                        lhsT=ht_s[:, c * 128:(c + 1) * 128],
                        rhs=w2p_t[:, :],
                        start=True, stop=True,
                    )

            def stageC(g, s, n):
                # batched softmax over one group (n blocks, na=[s..s+n) chunks)
                lg_ps = live.pop(("lg_ps", s // GRP))
                na = n * NCH
                ex_g = exgp.tile([128, GRP * NCH, NCLS], BF16, tag="exg")
                nc.scalar.activation(out=ex_g[:, 0:na, :],
                                     in_=lg_ps[:, 0:na, 0:NCLS],
                                     func=mybir.ActivationFunctionType.Exp)
                eb2_b = eb2_t[:, :].rearrange("p (a c) -> p a c", a=1)
                with nc.allow_low_precision(reason="softmax num in bf16"):
                    nc.vector.tensor_mul(ex_g[:, 0:na, :], ex_g[:, 0:na, :],
                                         eb2_b.broadcast_to([128, na, NCLS]))
                sum_g = recgp.tile([128, GRP * NCH], F32, tag="sumg")
                nc.vector.tensor_reduce(out=sum_g[:, 0:na], in_=ex_g[:, 0:na, :],
                                        axis=mybir.AxisListType.X,
                                        op=mybir.AluOpType.add)
                rec_g = recgp.tile([128, GRP * NCH], F32, tag="recg")
                nc.vector.reciprocal(out=rec_g[:, 0:na], in_=sum_g[:, 0:na])
                rec_b = rec_g[:, 0:na].rearrange("p (a c) -> p a c", c=1)
                out_v = out_t[:, s * NCH * NCLS:(s + n) * NCH * NCLS].rearrange(
                    "p (a c) -> p a c", c=NCLS)
                with nc.allow_low_precision(reason="probs in bf16"):
                    nc.vector.tensor_mul(out_v, ex_g[:, 0:na, :],
                                         rec_b.broadcast_to([128, na, NCLS]))

            def body():
                live.clear()
                fetch(0)
                if mode == "dma":
                    for B in range(1, nfg + 1):
                        fetch(B)
                    for B in range(nfg):
                        live.pop(("attr_g", B))
                        live.pop(("zz_g", B))
                    nc.gpsimd.dma_start(out=outE[:, :], in_=out_t[:])
                    return
                stageNR(0)
                for b in range(nblk + 3):
                    if b < nblk:
                        stageA(b)
                        if b + 1 < nblk:
                            stageNR(b + 1)
                    if 0 <= b - 1 < nblk:
                        stageB(b - 1)
                    if mode == "l1":
                        continue
                    gb = b - 3
                    if gb in sg_of_end:
                        stageC(*sg_of_end[gb])
                nc.gpsimd.dma_start(out=outE[:, :], in_=out_t[:])

            if reps == 1:
                body()
            else:
                with tc.For_i(0, reps, 1):
                    body()

    nc.compile()
    return nc


def _shard_inputs(z, edge_index, edge_attr, W1, b1, W2, b2):
    import ml_dtypes
    bf = ml_dtypes.bfloat16
    f8 = ml_dtypes.float8_e4m3
    z = np.asarray(z, dtype=np.float32)
    ei = np.asarray(edge_index).astype(np.int64)
    attr = np.asarray(edge_attr, dtype=np.float32)
    W1 = np.asarray(W1, dtype=np.float32)
    b1 = np.asarray(b1, dtype=np.float32)
    W2 = np.asarray(W2, dtype=np.float32)
    b2 = np.asarray(b2, dtype=np.float32)

    src = np.zeros(E_PAD, dtype=np.int64)
    dst = np.zeros(E_PAD, dtype=np.int64)
    src[:E_FULL] = ei[0]
    dst[:E_FULL] = ei[1]

    zb = z.astype(bf)
    # dense per-edge endpoint streams, feature-major, per-block [zs512|zd512]
    nblk_tot = E_PAD // BLK
    zzP = np.empty((ZD, nblk_tot, 2, BLK), dtype=bf)
    zzP[:, :, 0, :] = zb[src].T.reshape(ZD, nblk_tot, BLK)
    zzP[:, :, 1, :] = zb[dst].T.reshape(ZD, nblk_tot, BLK)
    zzP = zzP.reshape(ZD, 2 * E_PAD)

    # attr packed [128, nblk, 6, 512]: [p, b, s, e] = attr[b*512+e, s*128+p]
    attrP = np.zeros((AD, E_PAD), dtype=f8)
    attrP[:, :E_FULL] = attr.T.astype(f8)
    attrP = np.ascontiguousarray(
        attrP.reshape(NSL, 128, nblk_tot, BLK).transpose(1, 2, 0, 3))

    w1a = (W1[:ZD] * W1SC).astype(bf)             # [64, 128] node_rep rows
    w1f = np.ascontiguousarray(
        (W1[ZD:] * W1SC).reshape(NSL, 128, HID).transpose(1, 0, 2)).astype(f8)
    w2pad = np.zeros((HID, W2P), dtype=np.float32)
    w2pad[:, :NCLS] = W2
    b1c = b1.reshape(HID, 1)
    # b2 enters multiplicatively after exp: pr = ex*e^b2 / sum(ex*e^b2)
    eb2 = np.broadcast_to(np.exp(b2).astype(bf), (128, NCLS)).copy()

    nb_core = NBLK
    in_maps = []
    for c in range(N_CORES):
        s = slice(c * nb_core, (c + 1) * nb_core)
        s2 = slice(2 * c * E_CORE, 2 * (c + 1) * E_CORE)
        in_maps.append({
            "attrP": np.ascontiguousarray(
                attrP[:, s]).reshape(128, nb_core * NSL * BLK),
            "zzP": np.ascontiguousarray(zzP[:, s2]),
            "w1a": w1a,
            "w1f": w1f,
            "w2p": w2pad,
            "b1": b1c,
            "eb2": eb2,
        })
    return in_maps


def _unshard_out(res, nblk=NBLK):
    # outE[c][p, b*20 + ch*5 + k] = prob(edge = c*E_CORE + b*512 + ch*128 + p,
    #                                    class k)
    outs = []
    for c in range(N_CORES):
        o = np.asarray(res[c]["outE"], dtype=np.float32)
        o = o.reshape(128, nblk, NCH, NCLS).transpose(1, 2, 0, 3)
        outs.append(o.reshape(nblk * BLK, NCLS))
    return np.concatenate(outs, axis=0)[:E_FULL]


def kernel(z, edge_index, edge_attr, W1, b1, W2, b2):
    in_maps = _shard_inputs(z, edge_index, edge_attr, W1, b1, W2, b2)
    nc = build_nc()
    res = run_bass_kernel_spmd(nc, in_maps, core_ids=list(range(N_CORES))).results
    return np.ascontiguousarray(_unshard_out(res))
